# revision 1
# baseline (speedup 1.0000x reference)
"""Trainium2 Bass kernel for the KSubspaceBaseModel objective.

Reference computes, for B=2048 samples x (B, D=1024) and subspace bases
Us (R=4, K=16, D, d=32):
    z = x @ U; x_ = z @ U^T; loss = 0.5*||x - x_||^2  (per b, r, k)
    obj_r = mean_b min_k loss

Algebraic collapse: with G = U^T U and L = chol(I - 0.5 G) folded host-side
(Ut = U @ L), loss = 0.5||x||^2 - ||Ut^T x||^2, so the device computes
z~ = Ut^T x, squares, sums each subspace's 32 latent columns, takes max_k.

Speed strategy vs the bf16 version:
  * fp8 e4m3 operands (Ut scaled by 4096 to stay in the normal range) with
    DoubleRow matmuls: each instruction consumes TWO 128-deep contraction
    chunks ([128, 2, M] stationary x [128, 2, N] moving), 2x PE throughput
    and half the DMA bytes.
  * ||x||^2 via the PE too: X^T X accumulated over all batch chunks in one
    [128,128] PSUM region; diag extracted with one DVE tensor_tensor_reduce
    against an identity matrix (diag[p] = sum_bc ||x_{bc*128+p}||^2).
  * group-major matmul order (a group = 512 output cols = one PSUM bank,
    accumulated over 4 kc-pairs) so each group's epilogue (ScalarE square,
    PoolE subspace-sum, DVE k-max) overlaps the next group's matmuls.
  * 3 parallel input HWDGE rings (sync=u0, scalar=xt, vector=u1); output
    staged as two DMAs so most of it leaves before the last group finishes.
"""

import numpy as np
import ml_dtypes

import concourse.bass as bass
import concourse.bacc as bacc
import concourse.mybir as mybir
import concourse.tile as tile
from concourse.bass_utils import run_bass_kernel_spmd

B, D, R, K, d = 2048, 1024, 4, 16, 32
NCORES = 8
NB = B // 4          # 512 samples per core
BC = NB // 128       # 4 batch chunks per core
NJ = 4               # contraction pairs (8 kc chunks, 2 per DoubleRow matmul)
SCALE = 4096.0       # Ut pre-scale so fp8 e4m3 values are normal-range
WARM = 5             # PE warm-up matmuls (p-state ramp during DMA head)

FP8 = mybir.dt.float8e4
BF16 = mybir.dt.bfloat16
FP32 = mybir.dt.float32

_COMPILED = {}
LAST_RESULTS = None


def _build():
    nc = bacc.Bacc("TRN2", target_bir_lowering=False, debug=False)
    # host pre-arranges every tensor into its exact SBUF image so each
    # partition's DMA read is one contiguous run
    xt = nc.dram_tensor("xt", [128, BC * NJ * 2 * 128], FP8,
                        kind="ExternalInput")        # [p, bc, j, i, b]
    u0 = nc.dram_tensor("u0", [128, NJ * 2 * 512], FP8, kind="ExternalInput")
    u1 = nc.dram_tensor("u1", [128, NJ * 2 * 512], FP8, kind="ExternalInput")
    idn = nc.dram_tensor("idn", [128, 128], FP32, kind="ExternalInput")
    outp = nc.dram_tensor("outp", [128, 9], FP32, kind="ExternalOutput")

    xt_v = xt.ap().rearrange("p (b j i n) -> p b j i n", b=BC, j=NJ, i=2)
    u_v = [u.ap().rearrange("p (j i c) -> p j i c", j=NJ, i=2)
           for u in (u0, u1)]
    DR = mybir.MatmulPerfMode.DoubleRow

    with tile.TileContext(nc) as tc:
        with (
            tc.tile_pool(name="xsb", bufs=1) as xpool,
            tc.tile_pool(name="usb", bufs=1) as upool,
            tc.tile_pool(name="esb", bufs=3) as epool,
            tc.tile_pool(name="asb", bufs=2) as apool,
            tc.tile_pool(name="single", bufs=1) as spool,
            tc.tile_pool(name="zp", bufs=1, space="PSUM") as zpool,
        ):
            # per-chunk tiles so the first matmul only waits on its own
            # chunk's DMA, not the whole tensor (priority-ordered rings:
            # sync=u0 by pair, scalar=x by batch chunk, gpsimd SWDGE=u1)
            xb = [xpool.tile([128, NJ, 2, 128], FP8, tag=f"x{bc}",
                             name=f"x{bc}") for bc in range(BC)]
            uj = [[upool.tile([128, 2, 512], FP8, tag=f"u{nh}_{j}",
                              name=f"u{nh}_{j}") for j in range(NJ)]
                  for nh in range(2)]
            id_t = spool.tile([128, 128], FP32, tag="idn", name="id_t")

            warm = spool.tile([128, 640], BF16, tag="warm")
            nc.gpsimd.memset(warm[:], 0.0)

            for j in range(NJ):
                nc.sync.dma_start(uj[0][j][:], u_v[0][:, j])
            for bc in range(BC):
                nc.scalar.dma_start(xb[bc][:], xt_v[:, bc])
            for j in range(NJ):
                nc.gpsimd.dma_start(uj[1][j][:], u_v[1][:, j])
            nc.sync.dma_start(id_t[:], idn.ap())

            # col 0 = diag(X^T X) = per-partition ||x||^2 summed over bc;
            # cols 1..8 = per-group k-maxes in emission order (nh outer)
            ostage_a = spool.tile([128, 7], FP32, tag="oa", name="ostage_a")
            ostage_b = spool.tile([128, 2], FP32, tag="ob", name="ostage_b")

            # xq bank hosts the warm-up writes, then the X^T X accumulation
            # in its first 128 columns (same tile -> WAW-ordered by Tile)
            xq = zpool.tile([128, 512], FP32, tag="xq", name="xq")
            for _ in range(WARM):
                nc.tensor.matmul(xq[:], warm[:, 0:128], warm[:, 128:640],
                                 start=True, stop=True)

            # 4 PSUM banks, shared between the two nh halves (WAR on the
            # group's square, 4 groups of slack before reuse)
            zps = {bc: zpool.tile([128, 512], FP32, tag=f"zp_{bc}",
                                  name=f"zp_{bc}") for bc in range(BC)}

            gi = 0
            for nh in range(2):
                for bc in range(BC):
                    # moving operand is HW-capped at 512 streamed columns,
                    # so each group runs as two serial 256-col accumulations
                    e = epool.tile([128, 512], BF16, tag="e")
                    for half in range(2):
                        sl = slice(half * 256, (half + 1) * 256)
                        for j in range(NJ):
                            nc.tensor.matmul(
                                zps[bc][:, sl], xb[bc][:, j],
                                uj[nh][j][:, :, sl],
                                start=(j == 0), stop=(j == NJ - 1),
                                perf_mode=DR, skip_group_check=True)
                        nc.scalar.square(e[:, sl], zps[bc][:, sl])
                    if nh == 0:
                        for j in range(NJ):
                            nc.tensor.matmul(
                                xq[:, 0:128], xb[bc][:, j], xb[bc][:, j],
                                start=(bc == 0 and j == 0),
                                stop=(bc == BC - 1 and j == NJ - 1),
                                perf_mode=DR, skip_group_check=True)
                    a = apool.tile([128, K], FP32, tag="a")
                    nc.vector.reduce_sum(
                        a[:], e.rearrange("p (k c) -> p k c", c=d),
                        axis=mybir.AxisListType.X)
                    col = 1 + gi
                    dst, dcol = (ostage_a, col) if col < 7 else \
                                (ostage_b, col - 7)
                    nc.vector.reduce_max(dst[:, dcol:dcol + 1], a[:],
                                         axis=mybir.AxisListType.X)
                    if nh == 0 and bc == BC - 1:
                        # diag(X^T X) -> ostage col 0 (mask w/ identity, sum)
                        scr = spool.tile([128, 128], FP32, tag="scr")
                        nc.vector.tensor_mul(scr[:], xq[:, 0:128], id_t[:])
                        nc.vector.reduce_sum(ostage_a[:, 0:1], scr[:],
                                             axis=mybir.AxisListType.X)
                    if gi == 5:
                        nc.sync.dma_start(outp.ap()[:, 0:7], ostage_a[:])
                    gi += 1
            nc.sync.dma_start(outp.ap()[:, 7:9], ostage_b[:])

    nc.compile()
    return nc


def _prep(x, Us):
    # fold chol(I - 0.5 U^T U) into U, then scale+quantize to fp8 e4m3
    Us64 = Us.astype(np.float64)
    G = np.einsum('skDa,skDb->skab', Us64, Us64)
    L = np.linalg.cholesky(np.eye(d)[None, None] - 0.5 * G)
    Ut = np.einsum('skDa,skab->skDb', Us64, L)                # (R,K,D,d)
    u8 = (Ut * SCALE).astype(np.float32).astype(ml_dtypes.float8_e4m3)
    x8 = np.ascontiguousarray(x.T).astype(ml_dtypes.float8_e4m3)  # (D, B)
    ident = np.eye(128, dtype=np.float32)

    def u_img(r):  # one replicate -> [128, NJ*2*512] (p, j, i, c)
        ur = np.ascontiguousarray(u8[r].transpose(1, 0, 2)).reshape(D, K * d)
        return np.ascontiguousarray(
            ur.reshape(NJ, 2, 128, K * d).transpose(2, 0, 1, 3)
        ).reshape(128, NJ * 2 * K * d)

    def x_img(b4):  # one batch quarter -> [128, BC*NJ*2*128] (p, bc, j, i, n)
        xc = x8[:, NB * b4: NB * (b4 + 1)]                    # (D, 512)
        return np.ascontiguousarray(
            xc.reshape(NJ, 2, 128, BC, 128).transpose(2, 3, 0, 1, 4)
        ).reshape(128, BC * NJ * 2 * 128)

    u_imgs = [u_img(r) for r in range(R)]
    x_imgs = [x_img(b4) for b4 in range(BC)]
    in_maps = []
    for c in range(NCORES):
        s2, b4 = c // 4, c % 4
        in_maps.append({
            "xt": x_imgs[b4],
            "u0": u_imgs[2 * s2],
            "u1": u_imgs[2 * s2 + 1],
            "idn": ident,
        })
    return in_maps


def kernel(x, Us, _trace=False):
    global LAST_RESULTS
    if "nc" not in _COMPILED:
        _COMPILED["nc"] = _build()
    nc = _COMPILED["nc"]
    in_maps = _prep(np.asarray(x), np.asarray(Us))
    res = run_bass_kernel_spmd(nc, in_maps, core_ids=list(range(NCORES)),
                               trace=_trace)
    LAST_RESULTS = res
    # x is batch-sharded over cores 0..3 (cores 4..7 hold the same quarters)
    S = sum(res.results[c]["outp"][:, 0].astype(np.float64).sum()
            for c in range(4))
    base = 0.5 * S / B
    obj = np.empty(R, np.float32)
    for r in range(R):
        s2, nh = r // 2, r % 2
        # group col = 1 + nh*4 + bc; z~ was scaled by SCALE
        cols = [res.results[4 * s2 + b4]["outp"][:, 1 + nh * 4 + bc]
                for b4 in range(4) for bc in range(BC)]
        term = np.mean(np.stack(cols).astype(np.float64)) / (SCALE * SCALE)
        obj[r] = np.float32(base - term)
    return obj

